# revision 13
# baseline (speedup 1.0000x reference)
"""ConvCBAM Trainium2 kernel: depthwise conv1d + channels-last LN + MLP(18->72->18,
exact GELU) + layer-scale + CBAM channel attention + residual GELU.

Sharding: pure data-parallel, batch 16 -> 8 cores x 2. Layout per batch:
[126 partitions = 7 L-groups x 18 channels, Lsub=18944 cols] (+6 halo, +ones row).
The halo'd group layout (incl. zero padding and the ones row) is packed on the
host into one contiguous [127, HALO] array per batch so a SINGLE DMA loads it --
many small DMA producers per SBUF tile blow the per-instruction sync-wait budget
at the first conv Matmult ("Too many sync wait commands" in walrus).

Numerics notes (hardware-verified):
- TensorScalarPtr honors only ONE pointer scalar: (in op0 s1) with op1+scalar2
  silently dropped, and accum_out writes zeros. gamma (layer scale) is therefore
  folded into the m2 weights/bias host-side and tsums uses tensor_reduce.
- Input x and output y travel as bf16 (the axon PJRT tunnel moves ~30-50 MB/s,
  so wire bytes dominate wall time); LN/MLP internals stay f32/f32r. Worst-case
  added error ~1% of max|y|, well under the 2e-2 gate.

Execution: a module-cached jit (shard_map over 8 cores) built once per process;
replicated consts live on-device across calls, and the donated output buffers
are created on-device by a tiny jitted zeros-maker instead of shipping 150 MB
of host zeros per call like run_bass_kernel_spmd's helper does.
"""
import time
import numpy as np
import ml_dtypes

BF16 = ml_dtypes.bfloat16

B, C, L = 16, 18, 131072
K = 7
G = 7                 # L-groups per batch
P = G * C             # 126 data partitions
LSUB = 18944          # 37 * 512; G*LSUB = 132608 >= L
W = 512
NT = LSUB // W        # 37 tiles
VAL6 = L - 6 * LSUB   # 17408 valid cols in group 6 (= 34 tiles)
NTFULL = VAL6 // W    # 34 tiles where all 7 groups valid
HALO = LSUB + 6
XL = G * LSUB + 6     # padded length covered by the group windows
EPS = 1e-6
NCORES = 8
BPC = B // NCORES     # batches per core

_CACHE = {}
TIMINGS = {}
LAST_PATH = None


def _build_real(consts_np):
    import concourse.bacc as bacc
    import concourse.mybir as mybir
    import concourse.tile as tile

    f32 = mybir.dt.float32
    f32r = mybir.dt.float32r
    bf16 = mybir.dt.bfloat16
    Alu = mybir.AluOpType
    Act = mybir.ActivationFunctionType
    Ax = mybir.AxisListType

    # Bacc (not plain Bass): its compile() runs move_matmul_waits_to_ldweights
    # + generate_event_semaphores, without which walrus codegen rejects any
    # instruction carrying >1 sync wait ("Too many sync wait commands").
    nc = bacc.Bacc(debug=False)
    xg = nc.dram_tensor("xg", [BPC, 127, HALO], bf16, kind="ExternalInput")
    y2 = nc.dram_tensor("y2", [BPC, C, L], bf16, kind="ExternalOutput")
    cst = {}
    for name, arr in consts_np.items():
        cst[name] = nc.dram_tensor(
            name, list(arr.shape), mybir.dt.from_np(arr.dtype), kind="ExternalInput"
        )

    with tile.TileContext(nc) as tc:
        with (
            tc.tile_pool(name="big", bufs=1) as big,
            tc.tile_pool(name="zp", bufs=2) as zp,
            tc.tile_pool(name="hp", bufs=1) as hp,
            tc.tile_pool(name="op", bufs=3) as op_pool,
            tc.tile_pool(name="sm", bufs=1) as smp,
            tc.tile_pool(name="psZ", bufs=1, space="PSUM") as psZ,
            tc.tile_pool(name="psMU", bufs=1, space="PSUM") as psMU,
            tc.tile_pool(name="psVR", bufs=1, space="PSUM") as psVR,
            tc.tile_pool(name="psRS", bufs=1, space="PSUM") as psRS,
            tc.tile_pool(name="psH", bufs=1, space="PSUM") as psH,
            tc.tile_pool(name="psZ2", bufs=1, space="PSUM") as psZ2,
            tc.tile_pool(name="dram", bufs=1, space="DRAM") as dram,
        ):
            cw = big.tile([127, 7 * 126], bf16)
            nc.sync.dma_start(cw[:], cst["cw"][:])
            mb = big.tile([126, 126], f32r)
            nc.sync.dma_start(mb[:], cst["mb"][:].bitcast(f32r))
            vn = big.tile([126, 7], f32r)
            nc.sync.dma_start(vn[:], cst["vn"][:].bitcast(f32r))
            bc = big.tile([7, 126], f32)
            nc.sync.dma_start(bc[:], cst["bc"][:])
            m1 = big.tile([126, 4 * 126], f32r)
            nc.sync.dma_start(m1[:], cst["m1"][:].bitcast(f32r))
            m2 = big.tile([126, 4 * 126], f32r)
            nc.sync.dma_start(m2[:], cst["m2"][:].bitcast(f32r))
            b1c = big.tile([126, 4], f32)
            nc.sync.dma_start(b1c[:], cst["b1c"][:])
            b2v = big.tile([126, 1], f32)
            nc.sync.dma_start(b2v[:], cst["b2v"][:])
            caw1 = big.tile([18, 1], f32)
            nc.sync.dma_start(caw1[:], cst["caw1"][:])
            caw2v = big.tile([126, 1], f32)
            nc.sync.dma_start(caw2v[:], cst["caw2v"][:])
            ones126 = big.tile([1, 126], f32)
            nc.sync.dma_start(ones126[:], cst["ones126"][:])
            epsb = big.tile([7, 1], f32)
            nc.vector.memset(epsb[:], EPS)

            xs = big.tile([127, HALO], bf16)
            tsb = big.tile([126, LSUB], bf16)
            tsums = smp.tile([126, NT], f32)
            tmaxs = smp.tile([126, NT], f32)
            sm = smp.tile([126, 2], f32)
            smt = smp.tile([18, 2, 7], f32)
            am = smp.tile([18, 2], f32)
            att = smp.tile([126, 1], f32)
            prel = smp.tile([1, 2], f32)
            s1 = smp.tile([1, 1], f32)
            dsc = dram.tile([126, 2], f32)

            for b in range(BPC):
                # ---- x load: host-packed [group*18+c (+ones row), col] w/ halo ----
                nc.sync.dma_start(xs[:, :], xg[b])

                # ---- pass 1 ----
                for i in range(NT):
                    w0 = i * W
                    Z = psZ.tile([126, W], f32)
                    for k in range(K):
                        nc.tensor.matmul(
                            Z[:], cw[:, k * 126:(k + 1) * 126],
                            xs[0:127, w0 + k: w0 + k + W],
                            start=(k == 0), stop=(k == 6),
                        )
                    zsb = zp.tile([126, W], f32r, tag="z")
                    nc.scalar.copy(zsb[:], Z[:])
                    MU = psMU.tile([126, W], f32)
                    nc.tensor.matmul(MU[:], mb[:], zsb[:])
                    zc = zp.tile([126, W], f32, tag="zc")
                    nc.vector.scalar_tensor_tensor(
                        zc[:], zsb[:].bitcast(f32), 1.0, MU[:], op0=Alu.mult, op1=Alu.subtract
                    )
                    zc2 = zp.tile([126, W], f32r, tag="zc2")
                    nc.scalar.square(zc2[:], zc[:])
                    VR = psVR.tile([7, W], f32)
                    nc.tensor.matmul(VR[:], vn[:], zc2[:])
                    sd = zp.tile([7, W], f32, tag="sd")
                    nc.scalar.activation(sd[:], VR[:], Act.Sqrt, bias=epsb[:], scale=1.0)
                    rs = zp.tile([7, W], f32, tag="rs")
                    nc.vector.reciprocal_approx_fast(out=rs[:], in_=sd[:])
                    RS = psRS.tile([126, W], f32)
                    nc.tensor.matmul(RS[:], bc[:], rs[:])
                    yln = zp.tile([126, W], f32r, tag="yln")
                    nc.vector.scalar_tensor_tensor(
                        yln[:], zc[:], 1.0, RS[:], op0=Alu.mult, op1=Alu.mult
                    )
                    Z2 = psZ2.tile([126, W], f32)
                    for half in range(2):
                        H = psH.tile([126, 2 * W], f32, tag="H")
                        hs = hp.tile([126, 2 * W], f32r, tag="hs")
                        for a2 in range(2):
                            a = half * 2 + a2
                            nc.tensor.matmul(
                                H[:, a2 * W:(a2 + 1) * W],
                                m1[:, a * 126:(a + 1) * 126], yln[:],
                            )
                            nc.scalar.activation(
                                hs[:, a2 * W:(a2 + 1) * W],
                                H[:, a2 * W:(a2 + 1) * W],
                                Act.Gelu, bias=b1c[:, a:a + 1], scale=1.0,
                            )
                        for a2 in range(2):
                            a = half * 2 + a2
                            nc.tensor.matmul(
                                Z2[:], m2[:, a * 126:(a + 1) * 126],
                                hs[:, a2 * W:(a2 + 1) * W],
                                start=(a == 0), stop=(a == 3),
                            )
                    # gamma is folded into m2/b2v host-side: HW TensorScalarPtr
                    # ignores a second pointer scalar (op1 never applies), and
                    # accum_out writes zeros -- use tensor_reduce instead.
                    nc.vector.tensor_scalar(
                        tsb[:, w0:w0 + W], Z2[:], b2v[:], None,
                        op0=Alu.add,
                    )
                    nc.vector.tensor_reduce(
                        tsums[:, i:i + 1], tsb[:, w0:w0 + W], axis=Ax.X, op=Alu.add
                    )
                    nc.vector.tensor_reduce(
                        tmaxs[:, i:i + 1], tsb[:, w0:w0 + W], axis=Ax.X, op=Alu.max
                    )

                # ---- channel attention ----
                nc.sync.dma_start(tsums[108:126, NTFULL:NT],
                                  cst["zeros18"][:, 0:NT - NTFULL])
                nc.sync.dma_start(tmaxs[108:126, NTFULL:NT],
                                  cst["neg18"][:, 0:NT - NTFULL])
                nc.vector.tensor_reduce(sm[:, 0:1], tsums[:], axis=Ax.X, op=Alu.add)
                nc.vector.tensor_reduce(sm[:, 1:2], tmaxs[:], axis=Ax.X, op=Alu.max)
                nc.sync.dma_start(dsc[:], sm[:])
                nc.sync.dma_start(
                    smt[:], dsc.tensor[:].rearrange("(j c) s -> c s j", c=18)
                )
                nc.vector.tensor_reduce(am[:, 0:1], smt[:, 0, :], axis=Ax.X, op=Alu.add)
                nc.vector.tensor_scalar(
                    am[:, 0:1], am[:, 0:1], 1.0 / L, None, op0=Alu.mult
                )
                nc.vector.tensor_reduce(am[:, 1:2], smt[:, 1, :], axis=Ax.X, op=Alu.max)
                PM = psVR.tile([1, 2], f32, tag="att")
                nc.tensor.matmul(PM[:], caw1[:], am[:])
                nc.vector.tensor_scalar(
                    prel[:], PM[:], 0.0, None, op0=Alu.max
                )
                nc.vector.tensor_reduce(s1[:], prel[:], axis=Ax.X, op=Alu.add)
                AB = psVR.tile([126, 1], f32, tag="att")
                nc.tensor.matmul(AB[:], ones126[:], s1[:])
                nc.scalar.activation(att[:], AB[:], Act.Sigmoid, scale=caw2v[:])

                # ---- pass 2: out = gelu(att*t + x) ----
                for i in range(NT):
                    w0 = i * W
                    rows = 126 if i < NTFULL else 108
                    pre = op_pool.tile([126, W], f32, tag="pre")
                    nc.vector.affine_then_add(
                        pre[0:rows, :], tsb[0:rows, w0:w0 + W],
                        xs[0:rows, 3 + w0: 3 + w0 + W],
                        scale=att[0:rows, :], bias=0.0,
                    )
                    osb = op_pool.tile([126, W], bf16, tag="osb")
                    nc.scalar.activation(osb[0:rows, :], pre[0:rows, :], Act.Gelu)
                    for j in range(7 if i < NTFULL else 6):
                        nc.sync.dma_start(
                            y2[b, :, j * LSUB + w0: j * LSUB + w0 + W],
                            osb[j * 18:(j + 1) * 18, :],
                        )
    nc.compile()
    return nc


def _consts(dw_w, dw_b, ln_w, ln_b, w1, b1, w2, b2, gamma, ca_w1, ca_w2):
    f = np.float32
    cw = np.zeros((7, 127, 126), f)
    for k in range(K):
        for j in range(G):
            for c in range(C):
                cw[k, j * 18 + c, j * 18 + c] = dw_w[c, 0, k]
        if k == 3:
            for j in range(G):
                for c in range(C):
                    cw[k, 126, j * 18 + c] = dw_b[c]
    mb = np.zeros((126, 126), f)
    vn = np.zeros((126, 7), f)
    bc = np.zeros((7, 126), f)
    for j in range(G):
        mb[j * 18:(j + 1) * 18, j * 18:(j + 1) * 18] = 1.0 / 18
        vn[j * 18:(j + 1) * 18, j] = 1.0 / 18
        bc[j, j * 18:(j + 1) * 18] = 1.0
    w1p = (w1 * ln_w[None, :]).astype(f)          # (72, 18)
    b1p = (b1 + w1 @ ln_b).astype(f)              # (72,)
    # layer-scale gamma folded into w2/b2 (t = gamma*(h@w2.T + b2)): the HW
    # TensorScalarPtr only honors one pointer scalar, so the kernel applies
    # just "+ b2v" and gamma must already live in the matmul weights.
    w2g = (gamma[:, None] * w2).astype(f)          # (18, 72)
    b2g = (gamma * b2).astype(f)                   # (18,)
    m1 = np.zeros((4, 126, 126), f)
    m2 = np.zeros((4, 126, 126), f)
    b1c = np.zeros((126, 4), f)
    for a in range(4):
        for j in range(G):
            m1[a, j * 18:(j + 1) * 18, j * 18:(j + 1) * 18] = w1p[a * 18:(a + 1) * 18, :].T
            m2[a, j * 18:(j + 1) * 18, j * 18:(j + 1) * 18] = w2g[:, a * 18:(a + 1) * 18].T
        for g in range(18):
            b1c[np.arange(G) * 18 + g, a] = b1p[a * 18 + g]
    tile18 = np.tile(np.arange(18), G)
    b2v = b2g[tile18].astype(f)[:, None]
    caw1 = ca_w1[0, :].astype(f)[:, None]         # (18,1)
    caw2v = ca_w2[tile18, 0].astype(f)[:, None]   # (126,1)
    cw = np.ascontiguousarray(cw.transpose(1, 0, 2).reshape(127, 7 * 126)).astype(BF16)
    m1 = np.ascontiguousarray(m1.transpose(1, 0, 2).reshape(126, 4 * 126))
    m2 = np.ascontiguousarray(m2.transpose(1, 0, 2).reshape(126, 4 * 126))
    return {
        "cw": cw, "mb": mb, "vn": vn, "bc": bc, "m1": m1, "m2": m2,
        "b1c": b1c, "b2v": b2v, "caw1": caw1, "caw2v": caw2v,
        "ones126": np.ones((1, 126), f),
        "zeros18": np.zeros((18, 8), f),
        "neg18": np.full((18, 8), -1e30, f),
    }


def _pack_xg(x):
    """[B, C, L] f32 -> [B, 127, HALO] bf16: 7 halo'd L-groups + ones row."""
    xb = x.astype(BF16)
    xpad = np.zeros((B, C, XL), BF16)
    xpad[:, :, 3:3 + L] = xb
    xgall = np.empty((B, 127, HALO), BF16)
    for j in range(G):
        xgall[:, j * C:(j + 1) * C, :] = xpad[:, :, j * LSUB:j * LSUB + HALO]
    xgall[:, 126, :] = 1.0
    return xgall


def _make_runner(nc):
    """Build the sharded jit callable + on-device zeros maker, once per process."""
    import jax
    import jax.numpy as jnp
    from jax.experimental.shard_map import shard_map
    from jax.sharding import Mesh, PartitionSpec, NamedSharding
    from concourse import bass2jax
    import concourse.mybir as mybir

    bass2jax.install_neuronx_cc_hook()
    assert nc.dbg_addr is None
    pid_name = nc.partition_id_tensor.name if nc.partition_id_tensor else None
    in_names, out_names, out_avals = [], [], []
    for alloc in nc.m.functions[0].allocations:
        if not isinstance(alloc, mybir.MemoryLocationSet):
            continue
        name = alloc.memorylocations[0].name
        if alloc.kind == "ExternalInput":
            if name != pid_name:
                in_names.append(name)
        elif alloc.kind == "ExternalOutput":
            assert alloc.tensor_shape is not None and alloc.dtype is not None
            out_names.append(name)
            out_avals.append(
                jax.core.ShapedArray(tuple(alloc.tensor_shape), mybir.dt.np(alloc.dtype))
            )
    n_in, n_out = len(in_names), len(out_names)
    all_names = in_names + out_names
    if pid_name is not None:
        all_names = all_names + [pid_name]
    all_names = tuple(all_names)

    def _body(*args):
        operands = list(args)
        if pid_name is not None:
            operands.append(bass2jax.partition_id_tensor())
        outs = bass2jax._bass_exec_p.bind(
            *operands,
            out_avals=tuple(out_avals),
            in_names=all_names,
            out_names=tuple(out_names),
            lowering_input_output_aliases=(),
            sim_require_finite=True,
            sim_require_nnan=True,
            nc=nc,
        )
        return tuple(outs)

    devices = jax.devices()[:NCORES]
    mesh = Mesh(np.asarray(devices), ("core",))
    spec = PartitionSpec("core")
    sharded = jax.jit(
        shard_map(_body, mesh=mesh, in_specs=(spec,) * (n_in + n_out),
                  out_specs=(spec,) * n_out, check_rep=False),
        donate_argnums=tuple(range(n_in, n_in + n_out)),
        keep_unused=True,
    )
    zshapes = [(NCORES * a.shape[0], *a.shape[1:]) for a in out_avals]
    zdtypes = [a.dtype for a in out_avals]
    sharding = NamedSharding(mesh, spec)
    zeros_maker = jax.jit(
        lambda: tuple(jnp.zeros(s, d) for s, d in zip(zshapes, zdtypes)),
        out_shardings=tuple([sharding] * n_out),
    )
    return {"fn": sharded, "in_names": in_names, "out_names": out_names,
            "sharding": sharding}


def _numpy_ref(x, dw_w, dw_b, ln_w, ln_b, w1, b1, w2, b2, gamma, ca_w1, ca_w2):
    try:
        from scipy.special import erf
    except Exception:
        import math
        erf = np.vectorize(math.erf)
    x = np.asarray(x, np.float64)
    pad = K // 2
    xp = np.pad(x, ((0, 0), (0, 0), (pad, pad)))
    y = np.zeros_like(x)
    for k in range(K):
        y += dw_w[None, :, 0, k:k + 1] * xp[:, :, k:k + L]
    y += dw_b[None, :, None]
    yt = y.transpose(0, 2, 1)
    mu = yt.mean(-1, keepdims=True)
    var = ((yt - mu) ** 2).mean(-1, keepdims=True)
    yt = (yt - mu) / np.sqrt(var + EPS) * ln_w + ln_b

    def gelu(v):
        return v * 0.5 * (1.0 + erf(v / np.sqrt(2.0)))

    h = gelu(yt @ w1.T + b1)
    yt = h @ w2.T + b2
    yt = gamma * yt
    y = yt.transpose(0, 2, 1)
    avg = y.mean(-1)
    mx = y.max(-1)

    def fc(v):
        return np.maximum(v @ ca_w1.T, 0) @ ca_w2.T

    att = 1.0 / (1.0 + np.exp(-(fc(avg) + fc(mx))))
    y = y * att[:, :, None]
    return gelu(y + x).astype(np.float32)


# zeros-maker is kept out of the runner dict so it is rebuilt cheaply; the
# donated buffers are consumed every call.
_ZEROS = {}


def kernel(**inputs):
    global LAST_PATH
    inputs = {k: np.asarray(v) for k, v in inputs.items()}
    x = inputs["x"].astype(np.float32)
    params = {k: inputs[k].astype(np.float32) for k in
              ("dw_w", "dw_b", "ln_w", "ln_b", "w1", "b1", "w2", "b2",
               "gamma", "ca_w1", "ca_w2")}
    try:
        import jax
        import jax.numpy as jnp
        t0 = time.perf_counter()
        consts = _consts(**params)
        if "nc" not in _CACHE:
            _CACHE["nc"] = _build_real(consts)
        nc = _CACHE["nc"]
        if "runner" not in _CACHE:
            _CACHE["runner"] = _make_runner(nc)
            r = _CACHE["runner"]
            # replicated consts, device-resident once
            _CACHE["constd"] = {
                name: jax.device_put(
                    np.concatenate([consts[name]] * NCORES, axis=0), r["sharding"]
                )
                for name in consts
            }
            _ZEROS["mk"] = jax.jit(
                lambda: (jnp.zeros((B, C, L), BF16),),
                out_shardings=(r["sharding"],),
            )
        r = _CACHE["runner"]
        constd = _CACHE["constd"]
        t1 = time.perf_counter()
        xgall = _pack_xg(x)
        t2 = time.perf_counter()
        args = []
        for name in r["in_names"]:
            if name == "xg":
                args.append(xgall)
            else:
                args.append(constd[name])
        zeros = _ZEROS["mk"]()
        outs = r["fn"](*args, *zeros)
        assert r["out_names"] == ["y2"]
        y = np.asarray(outs[0])
        t3 = time.perf_counter()
        y = y.astype(np.float32)
        t4 = time.perf_counter()
        TIMINGS.update(build=t1 - t0, pack=t2 - t1, run=t3 - t2, gather=t4 - t3)
        LAST_PATH = "bass"
        return y
    except Exception:
        import traceback
        traceback.print_exc()
        LAST_PATH = "numpy_fallback"
        return _numpy_ref(x, **params)


# revision 14
# speedup vs baseline: 1.0075x; 1.0075x over previous
"""ConvCBAM Trainium2 kernel: depthwise conv1d + channels-last LN + MLP(18->72->18,
exact GELU) + layer-scale + CBAM channel attention + residual GELU.

Sharding: pure data-parallel, batch 16 -> 8 cores x 2. Layout per batch:
[126 partitions = 7 L-groups x 18 channels, Lsub=18944 cols] (+6 halo, +ones row).
The halo'd group layout (incl. zero padding and the ones row) is packed on the
host into one contiguous [127, HALO] array per batch so a SINGLE DMA loads it --
many small DMA producers per SBUF tile blow the per-instruction sync-wait budget
at the first conv Matmult ("Too many sync wait commands" in walrus).

Numerics notes (hardware-verified):
- TensorScalarPtr honors only ONE pointer scalar: (in op0 s1) with op1+scalar2
  silently dropped, and accum_out writes zeros. gamma (layer scale) is therefore
  folded into the m2 weights/bias host-side and tsums uses tensor_reduce.
- Input x and output y travel as bf16 (the axon PJRT tunnel moves ~30-50 MB/s,
  so wire bytes dominate wall time); LN/MLP internals stay f32/f32r. Worst-case
  added error ~1% of max|y|, well under the 2e-2 gate.

Execution: a module-cached jit (shard_map over 8 cores) built once per process;
replicated consts live on-device across calls, and the donated output buffers
are created on-device by a tiny jitted zeros-maker instead of shipping 150 MB
of host zeros per call like run_bass_kernel_spmd's helper does.
"""
import time
import numpy as np
import ml_dtypes

BF16 = ml_dtypes.bfloat16

B, C, L = 16, 18, 131072
K = 7
G = 7                 # L-groups per batch
P = G * C             # 126 data partitions
LSUB = 18944          # 37 * 512; G*LSUB = 132608 >= L
W = 512
NT = LSUB // W        # 37 tiles
VAL6 = L - 6 * LSUB   # 17408 valid cols in group 6 (= 34 tiles)
NTFULL = VAL6 // W    # 34 tiles where all 7 groups valid
HALO = LSUB + 6
XL = G * LSUB + 6     # padded length covered by the group windows
EPS = 1e-6
NCORES = 8
BPC = B // NCORES     # batches per core

_CACHE = {}
TIMINGS = {}
LAST_PATH = None


def _build_real(consts_np):
    import concourse.bacc as bacc
    import concourse.mybir as mybir
    import concourse.tile as tile

    f32 = mybir.dt.float32
    f32r = mybir.dt.float32r
    bf16 = mybir.dt.bfloat16
    Alu = mybir.AluOpType
    Act = mybir.ActivationFunctionType
    Ax = mybir.AxisListType

    # Bacc (not plain Bass): its compile() runs move_matmul_waits_to_ldweights
    # + generate_event_semaphores, without which walrus codegen rejects any
    # instruction carrying >1 sync wait ("Too many sync wait commands").
    nc = bacc.Bacc(debug=False)
    xg = nc.dram_tensor("xg", [BPC, 127, HALO], bf16, kind="ExternalInput")
    y2 = nc.dram_tensor("y2", [BPC, C, L], bf16, kind="ExternalOutput")
    cst = {}
    for name, arr in consts_np.items():
        cst[name] = nc.dram_tensor(
            name, list(arr.shape), mybir.dt.from_np(arr.dtype), kind="ExternalInput"
        )

    with tile.TileContext(nc) as tc:
        with (
            tc.tile_pool(name="big", bufs=1) as big,
            tc.tile_pool(name="zp", bufs=2) as zp,
            tc.tile_pool(name="hp", bufs=1) as hp,
            tc.tile_pool(name="op", bufs=3) as op_pool,
            tc.tile_pool(name="sm", bufs=1) as smp,
            tc.tile_pool(name="psZ", bufs=1, space="PSUM") as psZ,
            tc.tile_pool(name="psMU", bufs=1, space="PSUM") as psMU,
            tc.tile_pool(name="psVR", bufs=1, space="PSUM") as psVR,
            tc.tile_pool(name="psRS", bufs=1, space="PSUM") as psRS,
            tc.tile_pool(name="psH", bufs=1, space="PSUM") as psH,
            tc.tile_pool(name="psZ2", bufs=1, space="PSUM") as psZ2,
            tc.tile_pool(name="dram", bufs=1, space="DRAM") as dram,
        ):
            cw = big.tile([127, 7 * 126], bf16)
            nc.sync.dma_start(cw[:], cst["cw"][:])
            mb = big.tile([126, 126], f32r)
            nc.sync.dma_start(mb[:], cst["mb"][:].bitcast(f32r))
            vn = big.tile([126, 7], f32r)
            nc.sync.dma_start(vn[:], cst["vn"][:].bitcast(f32r))
            bc = big.tile([7, 126], f32)
            nc.sync.dma_start(bc[:], cst["bc"][:])
            m1 = big.tile([126, 4 * 126], f32r)
            nc.sync.dma_start(m1[:], cst["m1"][:].bitcast(f32r))
            m2 = big.tile([126, 4 * 126], f32r)
            nc.sync.dma_start(m2[:], cst["m2"][:].bitcast(f32r))
            b1c = big.tile([126, 4], f32)
            nc.sync.dma_start(b1c[:], cst["b1c"][:])
            b2v = big.tile([126, 1], f32)
            nc.sync.dma_start(b2v[:], cst["b2v"][:])
            caw1 = big.tile([18, 1], f32)
            nc.sync.dma_start(caw1[:], cst["caw1"][:])
            caw2v = big.tile([126, 1], f32)
            nc.sync.dma_start(caw2v[:], cst["caw2v"][:])
            ones126 = big.tile([1, 126], f32)
            nc.sync.dma_start(ones126[:], cst["ones126"][:])
            epsb = big.tile([7, 1], f32)
            nc.vector.memset(epsb[:], EPS)

            xs = big.tile([127, HALO], bf16)
            tsb = big.tile([126, LSUB], bf16)
            tsums = smp.tile([126, NT], f32)
            tmaxs = smp.tile([126, NT], f32)
            sm = smp.tile([126, 2], f32)
            smt = smp.tile([18, 2, 7], f32)
            am = smp.tile([18, 2], f32)
            att = smp.tile([126, 1], f32)
            prel = smp.tile([1, 2], f32)
            s1 = smp.tile([1, 1], f32)
            dsc = dram.tile([126, 2], f32)

            for b in range(BPC):
                # ---- x load: host-packed [group*18+c (+ones row), col] w/ halo ----
                nc.sync.dma_start(xs[:, :], xg[b])

                # ---- pass 1 ----
                for i in range(NT):
                    w0 = i * W
                    Z = psZ.tile([126, W], f32)
                    for k in range(K):
                        nc.tensor.matmul(
                            Z[:], cw[:, k * 126:(k + 1) * 126],
                            xs[0:127, w0 + k: w0 + k + W],
                            start=(k == 0), stop=(k == 6),
                        )
                    zsb = zp.tile([126, W], f32r, tag="z")
                    nc.scalar.copy(zsb[:], Z[:])
                    MU = psMU.tile([126, W], f32)
                    nc.tensor.matmul(MU[:], mb[:], zsb[:])
                    zc = zp.tile([126, W], f32, tag="zc")
                    nc.vector.scalar_tensor_tensor(
                        zc[:], zsb[:].bitcast(f32), 1.0, MU[:], op0=Alu.mult, op1=Alu.subtract
                    )
                    zc2 = zp.tile([126, W], f32r, tag="zc2")
                    nc.scalar.square(zc2[:], zc[:])
                    VR = psVR.tile([7, W], f32)
                    nc.tensor.matmul(VR[:], vn[:], zc2[:])
                    sd = zp.tile([7, W], f32, tag="sd")
                    nc.scalar.activation(sd[:], VR[:], Act.Sqrt, bias=epsb[:], scale=1.0)
                    rs = zp.tile([7, W], f32, tag="rs")
                    nc.vector.reciprocal_approx_fast(out=rs[:], in_=sd[:])
                    RS = psRS.tile([126, W], f32)
                    nc.tensor.matmul(RS[:], bc[:], rs[:])
                    yln = zp.tile([126, W], f32r, tag="yln")
                    nc.vector.scalar_tensor_tensor(
                        yln[:], zc[:], 1.0, RS[:], op0=Alu.mult, op1=Alu.mult
                    )
                    Z2 = psZ2.tile([126, W], f32)
                    for half in range(2):
                        H = psH.tile([126, 2 * W], f32, tag="H")
                        hs = hp.tile([126, 2 * W], f32r, tag="hs")
                        for a2 in range(2):
                            a = half * 2 + a2
                            nc.tensor.matmul(
                                H[:, a2 * W:(a2 + 1) * W],
                                m1[:, a * 126:(a + 1) * 126], yln[:],
                            )
                            nc.scalar.activation(
                                hs[:, a2 * W:(a2 + 1) * W],
                                H[:, a2 * W:(a2 + 1) * W],
                                Act.Gelu, bias=b1c[:, a:a + 1], scale=1.0,
                            )
                        for a2 in range(2):
                            a = half * 2 + a2
                            nc.tensor.matmul(
                                Z2[:], m2[:, a * 126:(a + 1) * 126],
                                hs[:, a2 * W:(a2 + 1) * W],
                                start=(a == 0), stop=(a == 3),
                            )
                    # gamma is folded into m2/b2v host-side: HW TensorScalarPtr
                    # ignores a second pointer scalar (op1 never applies), and
                    # accum_out writes zeros -- use tensor_reduce instead.
                    nc.vector.tensor_scalar(
                        tsb[:, w0:w0 + W], Z2[:], b2v[:], None,
                        op0=Alu.add,
                    )
                    nc.vector.tensor_reduce(
                        tsums[:, i:i + 1], tsb[:, w0:w0 + W], axis=Ax.X, op=Alu.add
                    )
                    nc.vector.tensor_reduce(
                        tmaxs[:, i:i + 1], tsb[:, w0:w0 + W], axis=Ax.X, op=Alu.max
                    )

                # ---- channel attention ----
                nc.sync.dma_start(tsums[108:126, NTFULL:NT],
                                  cst["zeros18"][:, 0:NT - NTFULL])
                nc.sync.dma_start(tmaxs[108:126, NTFULL:NT],
                                  cst["neg18"][:, 0:NT - NTFULL])
                nc.vector.tensor_reduce(sm[:, 0:1], tsums[:], axis=Ax.X, op=Alu.add)
                nc.vector.tensor_reduce(sm[:, 1:2], tmaxs[:], axis=Ax.X, op=Alu.max)
                nc.sync.dma_start(dsc[:], sm[:])
                nc.sync.dma_start(
                    smt[:], dsc.tensor[:].rearrange("(j c) s -> c s j", c=18)
                )
                nc.vector.tensor_reduce(am[:, 0:1], smt[:, 0, :], axis=Ax.X, op=Alu.add)
                nc.vector.tensor_scalar(
                    am[:, 0:1], am[:, 0:1], 1.0 / L, None, op0=Alu.mult
                )
                nc.vector.tensor_reduce(am[:, 1:2], smt[:, 1, :], axis=Ax.X, op=Alu.max)
                PM = psVR.tile([1, 2], f32, tag="att")
                nc.tensor.matmul(PM[:], caw1[:], am[:])
                nc.vector.tensor_scalar(
                    prel[:], PM[:], 0.0, None, op0=Alu.max
                )
                nc.vector.tensor_reduce(s1[:], prel[:], axis=Ax.X, op=Alu.add)
                AB = psVR.tile([126, 1], f32, tag="att")
                nc.tensor.matmul(AB[:], ones126[:], s1[:])
                nc.scalar.activation(att[:], AB[:], Act.Sigmoid, scale=caw2v[:])

                # ---- pass 2: out = gelu(att*t + x) ----
                for i in range(NT):
                    w0 = i * W
                    rows = 126 if i < NTFULL else 108
                    pre = op_pool.tile([126, W], f32, tag="pre")
                    nc.vector.affine_then_add(
                        pre[0:rows, :], tsb[0:rows, w0:w0 + W],
                        xs[0:rows, 3 + w0: 3 + w0 + W],
                        scale=att[0:rows, :], bias=0.0,
                    )
                    osb = op_pool.tile([126, W], bf16, tag="osb")
                    nc.scalar.activation(osb[0:rows, :], pre[0:rows, :], Act.Gelu)
                    for j in range(7 if i < NTFULL else 6):
                        nc.sync.dma_start(
                            y2[b, :, j * LSUB + w0: j * LSUB + w0 + W],
                            osb[j * 18:(j + 1) * 18, :],
                        )
    nc.compile()
    return nc


def _consts(dw_w, dw_b, ln_w, ln_b, w1, b1, w2, b2, gamma, ca_w1, ca_w2):
    f = np.float32
    cw = np.zeros((7, 127, 126), f)
    for k in range(K):
        for j in range(G):
            for c in range(C):
                cw[k, j * 18 + c, j * 18 + c] = dw_w[c, 0, k]
        if k == 3:
            for j in range(G):
                for c in range(C):
                    cw[k, 126, j * 18 + c] = dw_b[c]
    mb = np.zeros((126, 126), f)
    vn = np.zeros((126, 7), f)
    bc = np.zeros((7, 126), f)
    for j in range(G):
        mb[j * 18:(j + 1) * 18, j * 18:(j + 1) * 18] = 1.0 / 18
        vn[j * 18:(j + 1) * 18, j] = 1.0 / 18
        bc[j, j * 18:(j + 1) * 18] = 1.0
    w1p = (w1 * ln_w[None, :]).astype(f)          # (72, 18)
    b1p = (b1 + w1 @ ln_b).astype(f)              # (72,)
    # layer-scale gamma folded into w2/b2 (t = gamma*(h@w2.T + b2)): the HW
    # TensorScalarPtr only honors one pointer scalar, so the kernel applies
    # just "+ b2v" and gamma must already live in the matmul weights.
    w2g = (gamma[:, None] * w2).astype(f)          # (18, 72)
    b2g = (gamma * b2).astype(f)                   # (18,)
    m1 = np.zeros((4, 126, 126), f)
    m2 = np.zeros((4, 126, 126), f)
    b1c = np.zeros((126, 4), f)
    for a in range(4):
        for j in range(G):
            m1[a, j * 18:(j + 1) * 18, j * 18:(j + 1) * 18] = w1p[a * 18:(a + 1) * 18, :].T
            m2[a, j * 18:(j + 1) * 18, j * 18:(j + 1) * 18] = w2g[:, a * 18:(a + 1) * 18].T
        for g in range(18):
            b1c[np.arange(G) * 18 + g, a] = b1p[a * 18 + g]
    tile18 = np.tile(np.arange(18), G)
    b2v = b2g[tile18].astype(f)[:, None]
    caw1 = ca_w1[0, :].astype(f)[:, None]         # (18,1)
    caw2v = ca_w2[tile18, 0].astype(f)[:, None]   # (126,1)
    cw = np.ascontiguousarray(cw.transpose(1, 0, 2).reshape(127, 7 * 126)).astype(BF16)
    m1 = np.ascontiguousarray(m1.transpose(1, 0, 2).reshape(126, 4 * 126))
    m2 = np.ascontiguousarray(m2.transpose(1, 0, 2).reshape(126, 4 * 126))
    return {
        "cw": cw, "mb": mb, "vn": vn, "bc": bc, "m1": m1, "m2": m2,
        "b1c": b1c, "b2v": b2v, "caw1": caw1, "caw2v": caw2v,
        "ones126": np.ones((1, 126), f),
        "zeros18": np.zeros((18, 8), f),
        "neg18": np.full((18, 8), -1e30, f),
    }


def _pack_xg(x):
    """[B, C, L] f32 -> [B, 127, HALO] bf16: 7 halo'd L-groups + ones row."""
    xb = x.astype(BF16)
    xpad = np.zeros((B, C, XL), BF16)
    xpad[:, :, 3:3 + L] = xb
    xgall = np.empty((B, 127, HALO), BF16)
    for j in range(G):
        xgall[:, j * C:(j + 1) * C, :] = xpad[:, :, j * LSUB:j * LSUB + HALO]
    xgall[:, 126, :] = 1.0
    return xgall


def _make_runner(nc):
    """Build the sharded jit callable + on-device zeros maker, once per process."""
    import jax
    import jax.numpy as jnp
    from jax.experimental.shard_map import shard_map
    from jax.sharding import Mesh, PartitionSpec, NamedSharding
    from concourse import bass2jax
    import concourse.mybir as mybir

    bass2jax.install_neuronx_cc_hook()
    assert nc.dbg_addr is None
    pid_name = nc.partition_id_tensor.name if nc.partition_id_tensor else None
    in_names, out_names, out_avals = [], [], []
    for alloc in nc.m.functions[0].allocations:
        if not isinstance(alloc, mybir.MemoryLocationSet):
            continue
        name = alloc.memorylocations[0].name
        if alloc.kind == "ExternalInput":
            if name != pid_name:
                in_names.append(name)
        elif alloc.kind == "ExternalOutput":
            assert alloc.tensor_shape is not None and alloc.dtype is not None
            out_names.append(name)
            out_avals.append(
                jax.core.ShapedArray(tuple(alloc.tensor_shape), mybir.dt.np(alloc.dtype))
            )
    n_in, n_out = len(in_names), len(out_names)
    all_names = in_names + out_names
    if pid_name is not None:
        all_names = all_names + [pid_name]
    all_names = tuple(all_names)

    def _body(*args):
        operands = list(args)
        if pid_name is not None:
            operands.append(bass2jax.partition_id_tensor())
        outs = bass2jax._bass_exec_p.bind(
            *operands,
            out_avals=tuple(out_avals),
            in_names=all_names,
            out_names=tuple(out_names),
            lowering_input_output_aliases=(),
            sim_require_finite=True,
            sim_require_nnan=True,
            nc=nc,
        )
        return tuple(outs)

    devices = jax.devices()[:NCORES]
    mesh = Mesh(np.asarray(devices), ("core",))
    spec = PartitionSpec("core")
    sharded = jax.jit(
        shard_map(_body, mesh=mesh, in_specs=(spec,) * (n_in + n_out),
                  out_specs=(spec,) * n_out, check_rep=False),
        donate_argnums=tuple(range(n_in, n_in + n_out)),
        keep_unused=True,
    )
    zshapes = [(NCORES * a.shape[0], *a.shape[1:]) for a in out_avals]
    zdtypes = [a.dtype for a in out_avals]
    sharding = NamedSharding(mesh, spec)
    zeros_maker = jax.jit(
        lambda: tuple(jnp.zeros(s, d) for s, d in zip(zshapes, zdtypes)),
        out_shardings=tuple([sharding] * n_out),
    )
    return {"fn": sharded, "in_names": in_names, "out_names": out_names,
            "sharding": sharding}


def _numpy_ref(x, dw_w, dw_b, ln_w, ln_b, w1, b1, w2, b2, gamma, ca_w1, ca_w2):
    try:
        from scipy.special import erf
    except Exception:
        import math
        erf = np.vectorize(math.erf)
    x = np.asarray(x, np.float64)
    pad = K // 2
    xp = np.pad(x, ((0, 0), (0, 0), (pad, pad)))
    y = np.zeros_like(x)
    for k in range(K):
        y += dw_w[None, :, 0, k:k + 1] * xp[:, :, k:k + L]
    y += dw_b[None, :, None]
    yt = y.transpose(0, 2, 1)
    mu = yt.mean(-1, keepdims=True)
    var = ((yt - mu) ** 2).mean(-1, keepdims=True)
    yt = (yt - mu) / np.sqrt(var + EPS) * ln_w + ln_b

    def gelu(v):
        return v * 0.5 * (1.0 + erf(v / np.sqrt(2.0)))

    h = gelu(yt @ w1.T + b1)
    yt = h @ w2.T + b2
    yt = gamma * yt
    y = yt.transpose(0, 2, 1)
    avg = y.mean(-1)
    mx = y.max(-1)

    def fc(v):
        return np.maximum(v @ ca_w1.T, 0) @ ca_w2.T

    att = 1.0 / (1.0 + np.exp(-(fc(avg) + fc(mx))))
    y = y * att[:, :, None]
    return gelu(y + x).astype(np.float32)


# zeros-maker is kept out of the runner dict so it is rebuilt cheaply; the
# donated buffers are consumed every call.
_ZEROS = {}


def kernel(**inputs):
    global LAST_PATH
    inputs = {k: np.asarray(v) for k, v in inputs.items()}
    x = inputs["x"].astype(np.float32)
    params = {k: inputs[k].astype(np.float32) for k in
              ("dw_w", "dw_b", "ln_w", "ln_b", "w1", "b1", "w2", "b2",
               "gamma", "ca_w1", "ca_w2")}
    try:
        import jax
        import jax.numpy as jnp
        t0 = time.perf_counter()
        consts = _consts(**params)
        if "nc" not in _CACHE:
            _CACHE["nc"] = _build_real(consts)
        nc = _CACHE["nc"]
        if "runner" not in _CACHE:
            _CACHE["runner"] = _make_runner(nc)
            r = _CACHE["runner"]
            # replicated consts, device-resident once
            _CACHE["constd"] = {
                name: jax.device_put(
                    np.concatenate([consts[name]] * NCORES, axis=0), r["sharding"]
                )
                for name in consts
            }
            _ZEROS["mk"] = jax.jit(
                lambda: (jnp.zeros((B, C, L), BF16),),
                out_shardings=(r["sharding"],),
            )
        r = _CACHE["runner"]
        constd = _CACHE["constd"]
        t1 = time.perf_counter()
        xgall = _pack_xg(x)
        t2 = time.perf_counter()
        args = []
        for name in r["in_names"]:
            if name == "xg":
                args.append(xgall)
            else:
                args.append(constd[name])
        zeros = _ZEROS["mk"]()
        outs = r["fn"](*args, *zeros)
        assert r["out_names"] == ["y2"]
        yb = np.asarray(outs[0])
        t3 = time.perf_counter()
        # exact bf16->f32: append 16 zero mantissa bits (faster than ml_dtypes astype)
        y = (yb.view(np.uint16).astype(np.uint32) << 16).view(np.float32)
        t4 = time.perf_counter()
        TIMINGS.update(build=t1 - t0, pack=t2 - t1, run=t3 - t2, gather=t4 - t3)
        LAST_PATH = "bass"
        return y
    except Exception:
        import traceback
        traceback.print_exc()
        LAST_PATH = "numpy_fallback"
        return _numpy_ref(x, **params)
